# revision 10
# baseline (speedup 1.0000x reference)
"""Single-head attention (B=8, S=2048, D=384) on 8 NeuronCores.

Sharding: data-parallel over batch — core b computes batch element b
entirely (the module is single-headed with no cross-batch coupling), all
three weight matrices replicated. Host marshalling is layout-only (zero
FLOPs): x pre-transposed/tiled to [128, 3, S], weights pre-tiled to
[128, 3, 384] (Wq/Wk natural [e,d]; Wv transposed [d,e]).

Per-core dataflow (f32 in/out, matmuls in float32r at full PE rate):
  - QK fold: scores = (x Wq^T)(x Wk^T)^T = x (Wq^T Wk) x^T. M = Wq^T Wk
    costs 9 tiny matmuls and replaces the separate Q and K projections
    with ONE projection TT = M^T x^T, saving ~15k PE cycles/core.
  - V = x Wv^T in natural [S, D] layout with two ones-columns appended.
  - alphaT[k, q] tiles = xT^T @ TT accumulated over 3 d-tiles; exp on
    ScalarE (|logit| < 60 so fp32 exp cannot overflow; softmax is
    shift-invariant so no max subtraction is needed).
  - PV and the softmax denominator accumulate together in one PSUM tile
    via the ones-columns (column D = sum_k exp); out = raw * recip(den).

v2 schedule (head/tail compression vs v1):
  - DMA head: wq rides the sync HWDGE ring and wk the scalar HWDGE ring
    concurrently (~184 GB/s each), so M = Wq^T Wk can start ~10us in.
    wv follows wk on scalar; x[:, :, 0:256] follows wq on sync; the
    remaining x streams on the gpsimd software DGE in 256-col chunks
    paced ahead of consumption.
  - Warm-up: constants are memset on VectorE (first engine ready), so
    dependency-free fp32 dummy matmuls start ~5us and f32r pads bridge
    to the first real matmul — the HAM clock gate is at 2.4 GHz before
    any real work issues.
  - chunk-0 attention interleaves the V projections (V(kt) right after
    PV(kt-2)) and the TT projections for chunks 1-3 (at kt 5/9/12), so
    the PE never waits for the x DMA tail.
  - Tail: per-chunk epilogue fans out across engines (recips on
    VectorE; scales on VectorE/ScalarE/GpSimdE; output DMA on
    sync/scalar HWDGE + gpsimd SW DGE), so the last chunk's epilogue is
    ~2.4us instead of ~8.7us of serialized post-processing.
"""

import os
import numpy as np

import concourse.bacc as bacc
import concourse.tile as tile
from concourse import mybir
from concourse import bass_utils

P = 128
S = 2048
D = 384
NB = 8
DT = D // P
ST = S // P
QC = 512
NQ = S // QC
F32 = mybir.dt.float32
F32R = mybir.dt.float32r

N_WARM_F32 = int(os.environ.get("ATT_WARM_F32", "5"))
WARM_PRE = int(os.environ.get("ATT_WARM_PRE", "3"))
WARM_MID = int(os.environ.get("ATT_WARM_MID", "6"))
TT_KTS = (5, 9, 12)
# attention q-chunks: (col_lo, width). The final 128-row chunk keeps the
# very last epilogue (recip+scale+store) tiny so the kernel tail is short.
CHUNKS = ((0, 512), (512, 512), (1024, 512), (1536, 384), (1920, 128))
# weight split point across the two HWDGE rings: the sync ring (q1) starts
# ~1.7us before the scalar ring (q10), so it carries more bytes.
WSPLIT = 240


def _build():
    nc = bacc.Bacc(
        "TRN2", target_bir_lowering=False, debug=False, enable_asserts=False
    )
    xt = nc.dram_tensor("xt", [P, DT, S], F32R, kind="ExternalInput").ap()
    wqn = nc.dram_tensor("wqn", [P, DT, D], F32R, kind="ExternalInput").ap()
    wkn = nc.dram_tensor("wkn", [P, DT, D], F32R, kind="ExternalInput").ap()
    wvt = nc.dram_tensor("wvt", [P, DT, D], F32R, kind="ExternalInput").ap()
    out = nc.dram_tensor("out", [S, D], F32, kind="ExternalOutput").ap()

    with tile.TileContext(nc) as tc:
        with (
            tc.tile_pool(name="const", bufs=1) as const_pool,
            tc.tile_pool(name="big", bufs=1) as big,
            tc.tile_pool(name="expool", bufs=4) as ex_pool,
            tc.tile_pool(name="obpool", bufs=4) as ob_pool,
            tc.tile_pool(name="smalls", bufs=4) as small_pool,
            tc.tile_pool(name="ps_stage", bufs=4, space="PSUM") as ps_stage,
            tc.tile_pool(name="ps_acc", bufs=4, space="PSUM") as ps_acc,
        ):
            xT = big.tile([P, DT, S], F32R, tag="xT", name="xT")
            tT = big.tile([P, DT, S], F32R, tag="tT", name="tT")
            vA = big.tile([P, ST, D + 2], F32R, tag="vA", name="vA")
            wqT = big.tile([P, DT, D], F32R, tag="wqT", name="wqT")
            wkT = big.tile([P, DT, D], F32R, tag="wkT", name="wkT")
            wvT = big.tile([P, DT, D], F32R, tag="wvT", name="wvT")
            mT = big.tile([P, DT, D], F32R, tag="mT", name="mT")

            # input DMAs first in per-engine program order. Byte-balanced
            # across the two HWDGE rings (sync=q1 starts ~1.7us before
            # scalar=q10): weights for M first, then x[0:512] for TT0, then
            # wv, then the x tail; the middle of x rides the SW DGE.
            ws = WSPLIT
            nc.sync.dma_start(out=wqT[:, :, 0:ws], in_=wqn[:, :, 0:ws])
            nc.sync.dma_start(out=wkT[:, :, 0:ws], in_=wkn[:, :, 0:ws])
            nc.sync.dma_start(out=xT[:, :, 0:256], in_=xt[:, :, 0:256])
            nc.sync.dma_start(out=wvT[:, :, 0:ws], in_=wvt[:, :, 0:ws])
            nc.sync.dma_start(
                out=xT[:, :, 1792:2048], in_=xt[:, :, 1792:2048]
            )
            nc.scalar.dma_start(out=wqT[:, :, ws:D], in_=wqn[:, :, ws:D])
            nc.scalar.dma_start(out=wkT[:, :, ws:D], in_=wkn[:, :, ws:D])
            nc.scalar.dma_start(out=xT[:, :, 256:512], in_=xt[:, :, 256:512])
            nc.scalar.dma_start(out=wvT[:, :, ws:D], in_=wvt[:, :, ws:D])
            nc.scalar.dma_start(
                out=xT[:, :, 1536:1792], in_=xt[:, :, 1536:1792]
            )
            # warm-up constants on GpSimdE (first engine out of the
            # framework preamble, ~5.9us) so the PE clock-gate warm-up
            # starts earliest; f32r tiles are filled by DVE casts
            ones_c = const_pool.tile([P, 2], F32, tag="ones", name="ones_c")
            warm_z = const_pool.tile([P, QC], F32, tag="warmz", name="warm_z")
            warm_w = const_pool.tile([P, P], F32R, tag="warmw", name="warm_w")
            warm_m = const_pool.tile([P, QC], F32R, tag="warmm", name="warm_m")
            nc.gpsimd.memset(ones_c, 1.0)
            nc.gpsimd.memset(warm_z, 0.0)
            for lo in range(512, 1536, 256):
                nc.gpsimd.dma_start(
                    out=xT[:, :, lo:lo + 256], in_=xt[:, :, lo:lo + 256]
                )
            nc.vector.tensor_copy(warm_m, warm_z)
            nc.vector.tensor_copy(warm_w, warm_z[:, 0:P])
            nc.vector.tensor_copy(
                vA[:, :, D:D + 2],
                ones_c.unsqueeze(1).broadcast_to([P, ST, 2]),
            )

            _proj_n = [0]

            def proj_tile():
                _proj_n[0] += 1
                return ps_stage.tile([P, QC], F32, tag="pj", name="pj")

            # PE prewarm: HAM clock gate needs ~3.4us of sustained PE
            # activity to unthrottle 1.2 -> 2.4 GHz. fp32 dummies (ones x
            # zeros) start ~5us; f32r pads bridge to the first real matmul.
            for _ in range(N_WARM_F32):
                pw = proj_tile()
                nc.tensor.matmul(
                    pw[0:2, :], ones_c, warm_z, start=True, stop=True
                )
            for _ in range(WARM_PRE):
                pw = proj_tile()
                nc.tensor.matmul(pw, warm_w, warm_m, start=True, stop=True)

            def compute_m():
                for dt_ in range(DT):
                    pm = proj_tile()
                    for et in range(DT):
                        nc.tensor.matmul(
                            pm[:, 0:D],
                            wqT[:, et, dt_ * P:(dt_ + 1) * P],
                            wkT[:, et, :],
                            start=(et == 0),
                            stop=(et == DT - 1),
                        )
                    nc.vector.tensor_copy(mT[:, dt_, :], pm[:, 0:D])

            def project_v(st):
                pv = proj_tile()
                for dt_ in range(DT):
                    nc.tensor.matmul(
                        pv[:, 0:D],
                        xT[:, dt_, st * P:(st + 1) * P],
                        wvT[:, dt_, :],
                        start=(dt_ == 0),
                        stop=(dt_ == DT - 1),
                    )
                nc.vector.tensor_copy(vA[:, st, 0:D], pv[:, 0:D])

            def project_t_chunk(qc, et):
                pp = proj_tile()
                for dt_ in range(DT):
                    nc.tensor.matmul(
                        pp,
                        mT[:, dt_, et * P:(et + 1) * P],
                        xT[:, dt_, qc * QC:(qc + 1) * QC],
                        start=(dt_ == 0),
                        stop=(dt_ == DT - 1),
                    )
                nc.vector.tensor_copy(tT[:, et, qc * QC:(qc + 1) * QC], pp)

            compute_m()
            for _ in range(WARM_MID):
                pw = proj_tile()
                nc.tensor.matmul(pw, warm_w, warm_m, start=True, stop=True)
            for et in range(DT):
                project_t_chunk(0, et)

            def emit_out(row, acc, slot, final):
                """Scale one 128-row output block and store it.

                slot rotates engines so consecutive blocks' epilogues run
                concurrently; the final block splits its store across both
                HWDGE rings to minimize the kernel tail.
                """
                rec = small_pool.tile([P, 1], F32, tag="rec", name="rec")
                ob = ob_pool.tile([P, D], F32, tag="ob", name="ob")
                nc.vector.reciprocal(rec, acc[:, D:D + 1])
                if slot % 2 == 0 or final:
                    nc.vector.tensor_scalar_mul(ob, acc[:, 0:D], rec)
                else:
                    nc.scalar.activation(
                        ob,
                        acc[:, 0:D],
                        mybir.ActivationFunctionType.Copy,
                        scale=rec,
                    )
                if final:
                    nc.sync.dma_start(
                        out=out[row:row + P, 0:D // 2], in_=ob[:, 0:D // 2]
                    )
                    nc.scalar.dma_start(
                        out=out[row:row + P, D // 2:D], in_=ob[:, D // 2:D]
                    )
                elif slot % 4 == 0:
                    nc.sync.dma_start(out=out[row:row + P, :], in_=ob)
                elif slot % 4 == 1:
                    nc.scalar.dma_start(out=out[row:row + P, :], in_=ob)
                elif slot % 4 == 2:
                    nc.gpsimd.dma_start(out=out[row:row + P, :], in_=ob)
                else:
                    nc.sync.dma_start(out=out[row:row + P, :], in_=ob)

            slot_n = [0]
            for ci, (qlo, qw) in enumerate(CHUNKS):
                nblk = qw // P
                accs = [
                    ps_acc.tile([P, D + 2], F32, tag="acc", name="acc")
                    for _ in range(nblk)
                ]

                def emit_pv(kt_i, ex):
                    for qs in range(nblk):
                        nc.tensor.matmul(
                            accs[qs],
                            ex[:, qs * P:(qs + 1) * P],
                            vA[:, kt_i, :],
                            start=(kt_i == 0),
                            stop=(kt_i == ST - 1),
                        )

                pending = []
                for kt_i in range(ST):
                    pa = ps_stage.tile([P, QC], F32, tag="pj", name="pa")
                    for et in range(DT):
                        nc.tensor.matmul(
                            pa[:, 0:qw],
                            xT[:, et, kt_i * P:(kt_i + 1) * P],
                            tT[:, et, qlo:qlo + qw],
                            start=(et == 0),
                            stop=(et == DT - 1),
                        )
                    ex = ex_pool.tile([P, QC], F32R, tag="ex", name="ex")
                    nc.scalar.activation(
                        ex[:, 0:qw], pa[:, 0:qw],
                        mybir.ActivationFunctionType.Exp,
                    )
                    pending.append((kt_i, ex))
                    if len(pending) > 2:
                        emit_pv(*pending.pop(0))
                    if ci == 0:
                        project_v(kt_i)
                        if kt_i in TT_KTS:
                            qc = TT_KTS.index(kt_i) + 1
                            for et in range(DT):
                                project_t_chunk(qc, et)
                for item in pending:
                    emit_pv(*item)

                is_last_chunk = ci == len(CHUNKS) - 1
                for qs in range(nblk):
                    final = is_last_chunk and qs == nblk - 1
                    emit_out(qlo + qs * P, accs[qs], slot_n[0], final)
                    slot_n[0] += 1

    nc.compile()
    return nc


_NC = None
_FAST = None


def _get_nc():
    global _NC
    if _NC is None:
        _NC = _build()
    return _NC


def _fast_runner():
    global _FAST
    if _FAST is not None:
        return _FAST
    import jax
    from jax.experimental.shard_map import shard_map
    from jax.sharding import Mesh, PartitionSpec

    from concourse import bass2jax

    nc = _get_nc()
    bass2jax.install_neuronx_cc_hook()

    in_names = ["xt", "wqn", "wkn", "wvt"]
    out_aval = jax.core.ShapedArray((S, D), np.float32)

    def _body(*args):
        operands = list(args)
        operands.append(bass2jax.partition_id_tensor())
        outs = bass2jax._bass_exec_p.bind(
            *operands,
            out_avals=(out_aval,),
            in_names=tuple(in_names) + ("out", "partition_id"),
            out_names=("out",),
            lowering_input_output_aliases=(),
            sim_require_finite=True,
            sim_require_nnan=True,
            nc=nc,
        )
        return tuple(outs)

    devices = jax.devices()[:NB]
    mesh = Mesh(np.asarray(devices), ("core",))
    n_in = len(in_names) + 1
    fn = jax.jit(
        shard_map(
            _body,
            mesh=mesh,
            in_specs=(PartitionSpec("core"),) * n_in,
            out_specs=(PartitionSpec("core"),),
            check_rep=False,
        ),
        donate_argnums=(n_in - 1,),
        keep_unused=True,
    )
    _FAST = fn
    return fn


def _tile_ed(w):
    return np.ascontiguousarray(
        w.reshape(DT, P, w.shape[1]).transpose(1, 0, 2)
    )


def _marshal(att_input, Wq, Wk, Wv):
    att_input = np.asarray(att_input, dtype=np.float32)
    xts = np.ascontiguousarray(
        att_input.transpose(0, 2, 1)
        .reshape(NB, DT, P, S)
        .transpose(0, 2, 1, 3)
    )
    wq = _tile_ed(np.asarray(Wq, dtype=np.float32))
    wk = _tile_ed(np.asarray(Wk, dtype=np.float32))
    wv = _tile_ed(np.ascontiguousarray(np.asarray(Wv, np.float32).T))
    return xts, (wq, wk, wv)


def run(att_input, Wq, Wk, Wv, trace=False):
    xts, wts = _marshal(att_input, Wq, Wk, Wv)
    if trace:
        in_maps = [
            {"xt": xts[b], "wqn": wts[0], "wkn": wts[1], "wvt": wts[2]}
            for b in range(NB)
        ]
        res = bass_utils.run_bass_kernel_spmd(
            _get_nc(), in_maps, core_ids=list(range(NB)), trace=True
        )
        out = np.stack([res.results[b]["out"] for b in range(NB)], axis=0)
        return out.astype(np.float32, copy=False), res

    try:
        fn = _fast_runner()
        xs = xts.reshape(NB * P, DT, S)
        ws = [
            np.concatenate([w] * NB, axis=0).reshape(NB * P, DT, D)
            for w in wts
        ]
        zeros = np.zeros((NB * S, D), np.float32)
        (out,) = fn(xs, *ws, zeros)
        out = np.asarray(out)
    except Exception:
        in_maps = [
            {"xt": xts[b], "wqn": wts[0], "wkn": wts[1], "wvt": wts[2]}
            for b in range(NB)
        ]
        res = bass_utils.run_bass_kernel_spmd(
            _get_nc(), in_maps, core_ids=list(range(NB))
        )
        out = np.stack([res.results[b]["out"] for b in range(NB)], axis=0)
    return out.reshape(NB, S, D).astype(np.float32, copy=False), None


def kernel(att_input, Wq, Wk, Wv):
    out, _ = run(att_input, Wq, Wk, Wv)
    return out


# revision 13
# speedup vs baseline: 1.0500x; 1.0500x over previous
"""Single-head attention (B=8, S=2048, D=384) on 8 NeuronCores.

Sharding: data-parallel over batch — core b computes batch element b
entirely (the module is single-headed with no cross-batch coupling), all
three weight matrices replicated. Host marshalling is layout-only (zero
FLOPs): x pre-transposed/tiled to [128, 3, S], weights pre-tiled to
[128, 3, 384] (Wq/Wk natural [e,d]; Wv transposed [d,e]).

Per-core dataflow (f32 in/out, matmuls in float32r at full PE rate):
  - QK fold: scores = (x Wq^T)(x Wk^T)^T = x (Wq^T Wk) x^T. M = Wq^T Wk
    costs 9 tiny matmuls and replaces the separate Q and K projections
    with ONE projection TT = M^T x^T, saving ~15k PE cycles/core.
  - V = x Wv^T in natural [S, D] layout with two ones-columns appended.
  - alphaT[k, q] tiles = xT^T @ TT accumulated over 3 d-tiles; exp on
    ScalarE (|logit| < 60 so fp32 exp cannot overflow; softmax is
    shift-invariant so no max subtraction is needed).
  - PV and the softmax denominator accumulate together in one PSUM tile
    via the ones-columns (column D = sum_k exp); out = raw * recip(den).

v2 schedule (head/tail compression vs v1):
  - DMA head: wq rides the sync HWDGE ring and wk the scalar HWDGE ring
    concurrently (~184 GB/s each), so M = Wq^T Wk can start ~10us in.
    wv follows wk on scalar; x[:, :, 0:256] follows wq on sync; the
    remaining x streams on the gpsimd software DGE in 256-col chunks
    paced ahead of consumption.
  - Warm-up: constants are memset on VectorE (first engine ready), so
    dependency-free fp32 dummy matmuls start ~5us and f32r pads bridge
    to the first real matmul — the HAM clock gate is at 2.4 GHz before
    any real work issues.
  - chunk-0 attention interleaves the V projections (V(kt) right after
    PV(kt-2)) and the TT projections for chunks 1-3 (at kt 5/9/12), so
    the PE never waits for the x DMA tail.
  - Tail: per-chunk epilogue fans out across engines (recips on
    VectorE; scales on VectorE/ScalarE/GpSimdE; output DMA on
    sync/scalar HWDGE + gpsimd SW DGE), so the last chunk's epilogue is
    ~2.4us instead of ~8.7us of serialized post-processing.
"""

import os
import numpy as np

import concourse.bacc as bacc
import concourse.tile as tile
from concourse import mybir
from concourse import bass_utils

P = 128
S = 2048
D = 384
NB = 8
DT = D // P
ST = S // P
QC = 512
NQ = S // QC
F32 = mybir.dt.float32
F32R = mybir.dt.float32r

N_WARM_F32 = int(os.environ.get("ATT_WARM_F32", "5"))
WARM_PRE = int(os.environ.get("ATT_WARM_PRE", "6"))
WARM_MID = int(os.environ.get("ATT_WARM_MID", "4"))
TT_KTS = (5, 9, 12)
# attention q-chunks: (col_lo, width). The final 128-row chunk keeps the
# very last epilogue (recip+scale+store) tiny so the kernel tail is short.
CHUNKS = ((0, 512), (512, 512), (1024, 512), (1536, 384), (1920, 128))


def _build():
    nc = bacc.Bacc(
        "TRN2", target_bir_lowering=False, debug=False, enable_asserts=False
    )
    xt = nc.dram_tensor("xt", [P, DT, S], F32R, kind="ExternalInput").ap()
    wqn = nc.dram_tensor("wqn", [P, DT, D], F32R, kind="ExternalInput").ap()
    wkn = nc.dram_tensor("wkn", [P, DT, D], F32R, kind="ExternalInput").ap()
    wvt = nc.dram_tensor("wvt", [P, DT, D], F32R, kind="ExternalInput").ap()
    out = nc.dram_tensor("out", [S, D], F32, kind="ExternalOutput").ap()

    with tile.TileContext(nc) as tc:
        with (
            tc.tile_pool(name="const", bufs=1) as const_pool,
            tc.tile_pool(name="big", bufs=1) as big,
            tc.tile_pool(name="expool", bufs=5) as ex_pool,
            tc.tile_pool(name="obpool", bufs=4) as ob_pool,
            tc.tile_pool(name="smalls", bufs=4) as small_pool,
            tc.tile_pool(name="ps_stage", bufs=4, space="PSUM") as ps_stage,
            tc.tile_pool(name="ps_acc", bufs=4, space="PSUM") as ps_acc,
        ):
            xT = big.tile([P, DT, S], F32R, tag="xT", name="xT")
            tT = big.tile([P, DT, S], F32R, tag="tT", name="tT")
            vA = big.tile([P, ST, D + 2], F32R, tag="vA", name="vA")
            wqT = big.tile([P, DT, D], F32R, tag="wqT", name="wqT")
            wkT = big.tile([P, DT, D], F32R, tag="wkT", name="wkT")
            wvT = big.tile([P, DT, D], F32R, tag="wvT", name="wvT")
            mT = big.tile([P, DT, D], F32R, tag="mT", name="mT")

            # input DMAs first in per-engine program order. HWDGE rings only
            # run fast with long contiguous runs, so weights go as whole
            # tensors (4.5KB runs): wq on sync (q1, starts ~1.7us before
            # scalar's q10), wk+wv on scalar; x rides partly on sync and
            # mostly on the SW DGE in fat chunks, paced ahead of use.
            nc.sync.dma_start(out=wqT, in_=wqn)
            nc.sync.dma_start(out=xT[:, :, 0:256], in_=xt[:, :, 0:256])
            nc.sync.dma_start(
                out=xT[:, :, 1536:2048], in_=xt[:, :, 1536:2048]
            )
            nc.scalar.dma_start(out=wkT, in_=wkn)
            nc.scalar.dma_start(out=wvT, in_=wvt)
            # warm-up constants on GpSimdE (first engine out of the
            # framework preamble, ~5.9us) so the PE clock-gate warm-up
            # starts earliest; f32r tiles are filled by DVE casts
            ones_c = const_pool.tile([P, 2], F32, tag="ones", name="ones_c")
            warm_z = const_pool.tile([P, QC], F32, tag="warmz", name="warm_z")
            warm_w = const_pool.tile([P, P], F32R, tag="warmw", name="warm_w")
            warm_m = const_pool.tile([P, QC], F32R, tag="warmm", name="warm_m")
            nc.gpsimd.memset(ones_c, 1.0)
            nc.gpsimd.memset(warm_z, 0.0)
            nc.gpsimd.dma_start(
                out=xT[:, :, 256:512], in_=xt[:, :, 256:512]
            )
            for lo in range(512, 1536, 512):
                nc.gpsimd.dma_start(
                    out=xT[:, :, lo:lo + 512], in_=xt[:, :, lo:lo + 512]
                )
            nc.vector.tensor_copy(warm_m, warm_z)
            nc.vector.tensor_copy(warm_w, warm_z[:, 0:P])
            nc.vector.tensor_copy(
                vA[:, :, D:D + 2],
                ones_c.unsqueeze(1).broadcast_to([P, ST, 2]),
            )

            _proj_n = [0]

            def proj_tile():
                _proj_n[0] += 1
                return ps_stage.tile([P, QC], F32, tag="pj", name="pj")

            # PE prewarm: HAM clock gate needs ~3.4us of sustained PE
            # activity to unthrottle 1.2 -> 2.4 GHz. fp32 dummies (ones x
            # zeros) start ~5us; f32r pads bridge to the first real matmul.
            for _ in range(N_WARM_F32):
                pw = proj_tile()
                nc.tensor.matmul(
                    pw[0:2, :], ones_c, warm_z, start=True, stop=True
                )
            for _ in range(WARM_PRE):
                pw = proj_tile()
                nc.tensor.matmul(pw, warm_w, warm_m, start=True, stop=True)

            def compute_m():
                for dt_ in range(DT):
                    pm = proj_tile()
                    for et in range(DT):
                        nc.tensor.matmul(
                            pm[:, 0:D],
                            wqT[:, et, dt_ * P:(dt_ + 1) * P],
                            wkT[:, et, :],
                            start=(et == 0),
                            stop=(et == DT - 1),
                        )
                    nc.vector.tensor_copy(mT[:, dt_, :], pm[:, 0:D])

            def project_v(st):
                pv = proj_tile()
                for dt_ in range(DT):
                    nc.tensor.matmul(
                        pv[:, 0:D],
                        xT[:, dt_, st * P:(st + 1) * P],
                        wvT[:, dt_, :],
                        start=(dt_ == 0),
                        stop=(dt_ == DT - 1),
                    )
                nc.vector.tensor_copy(vA[:, st, 0:D], pv[:, 0:D])

            def project_t_chunk(qc, et):
                pp = proj_tile()
                for dt_ in range(DT):
                    nc.tensor.matmul(
                        pp,
                        mT[:, dt_, et * P:(et + 1) * P],
                        xT[:, dt_, qc * QC:(qc + 1) * QC],
                        start=(dt_ == 0),
                        stop=(dt_ == DT - 1),
                    )
                nc.vector.tensor_copy(tT[:, et, qc * QC:(qc + 1) * QC], pp)

            compute_m()
            for _ in range(WARM_MID):
                pw = proj_tile()
                nc.tensor.matmul(pw, warm_w, warm_m, start=True, stop=True)
            for et in range(DT):
                project_t_chunk(0, et)

            def emit_out(row, acc, slot, final):
                """Scale one 128-row output block and store it.

                slot rotates engines so consecutive blocks' epilogues run
                concurrently; the final block splits its store across both
                HWDGE rings to minimize the kernel tail.
                """
                rec = small_pool.tile([P, 1], F32, tag="rec", name="rec")
                ob = ob_pool.tile([P, D], F32, tag="ob", name="ob")
                nc.vector.reciprocal(rec, acc[:, D:D + 1])
                if slot % 2 == 0 or final:
                    nc.vector.tensor_scalar_mul(ob, acc[:, 0:D], rec)
                else:
                    nc.scalar.activation(
                        ob,
                        acc[:, 0:D],
                        mybir.ActivationFunctionType.Copy,
                        scale=rec,
                    )
                if final:
                    nc.sync.dma_start(
                        out=out[row:row + P, 0:D // 2], in_=ob[:, 0:D // 2]
                    )
                    nc.scalar.dma_start(
                        out=out[row:row + P, D // 2:D], in_=ob[:, D // 2:D]
                    )
                elif slot % 4 == 0:
                    nc.sync.dma_start(out=out[row:row + P, :], in_=ob)
                elif slot % 4 == 1:
                    nc.scalar.dma_start(out=out[row:row + P, :], in_=ob)
                elif slot % 4 == 2:
                    nc.gpsimd.dma_start(out=out[row:row + P, :], in_=ob)
                else:
                    nc.sync.dma_start(out=out[row:row + P, :], in_=ob)

            slot_n = [0]
            for ci, (qlo, qw) in enumerate(CHUNKS):
                nblk = qw // P
                accs = [
                    ps_acc.tile([P, D + 2], F32, tag="acc", name="acc")
                    for _ in range(nblk)
                ]

                def emit_pv(kt_i, ex):
                    for qs in range(nblk):
                        nc.tensor.matmul(
                            accs[qs],
                            ex[:, qs * P:(qs + 1) * P],
                            vA[:, kt_i, :],
                            start=(kt_i == 0),
                            stop=(kt_i == ST - 1),
                        )

                pending = []
                for kt_i in range(ST):
                    pa = ps_stage.tile([P, QC], F32, tag="pj", name="pa")
                    for et in range(DT):
                        nc.tensor.matmul(
                            pa[:, 0:qw],
                            xT[:, et, kt_i * P:(kt_i + 1) * P],
                            tT[:, et, qlo:qlo + qw],
                            start=(et == 0),
                            stop=(et == DT - 1),
                        )
                    ex = ex_pool.tile([P, QC], F32R, tag="ex", name="ex")
                    nc.scalar.activation(
                        ex[:, 0:qw], pa[:, 0:qw],
                        mybir.ActivationFunctionType.Exp,
                    )
                    pending.append((kt_i, ex))
                    if len(pending) > 3:
                        emit_pv(*pending.pop(0))
                    if ci == 0:
                        project_v(kt_i)
                        if kt_i in TT_KTS:
                            qc = TT_KTS.index(kt_i) + 1
                            for et in range(DT):
                                project_t_chunk(qc, et)
                for item in pending:
                    emit_pv(*item)

                is_last_chunk = ci == len(CHUNKS) - 1
                for qs in range(nblk):
                    final = is_last_chunk and qs == nblk - 1
                    emit_out(qlo + qs * P, accs[qs], slot_n[0], final)
                    slot_n[0] += 1

    nc.compile()
    return nc


_NC = None
_FAST = None


def _get_nc():
    global _NC
    if _NC is None:
        _NC = _build()
    return _NC


def _fast_runner():
    global _FAST
    if _FAST is not None:
        return _FAST
    import jax
    from jax.experimental.shard_map import shard_map
    from jax.sharding import Mesh, PartitionSpec

    from concourse import bass2jax

    nc = _get_nc()
    bass2jax.install_neuronx_cc_hook()

    in_names = ["xt", "wqn", "wkn", "wvt"]
    out_aval = jax.core.ShapedArray((S, D), np.float32)

    def _body(*args):
        operands = list(args)
        operands.append(bass2jax.partition_id_tensor())
        outs = bass2jax._bass_exec_p.bind(
            *operands,
            out_avals=(out_aval,),
            in_names=tuple(in_names) + ("out", "partition_id"),
            out_names=("out",),
            lowering_input_output_aliases=(),
            sim_require_finite=True,
            sim_require_nnan=True,
            nc=nc,
        )
        return tuple(outs)

    devices = jax.devices()[:NB]
    mesh = Mesh(np.asarray(devices), ("core",))
    n_in = len(in_names) + 1
    fn = jax.jit(
        shard_map(
            _body,
            mesh=mesh,
            in_specs=(PartitionSpec("core"),) * n_in,
            out_specs=(PartitionSpec("core"),),
            check_rep=False,
        ),
        donate_argnums=(n_in - 1,),
        keep_unused=True,
    )
    _FAST = fn
    return fn


def _tile_ed(w):
    return np.ascontiguousarray(
        w.reshape(DT, P, w.shape[1]).transpose(1, 0, 2)
    )


def _marshal(att_input, Wq, Wk, Wv):
    att_input = np.asarray(att_input, dtype=np.float32)
    xts = np.ascontiguousarray(
        att_input.transpose(0, 2, 1)
        .reshape(NB, DT, P, S)
        .transpose(0, 2, 1, 3)
    )
    wq = _tile_ed(np.asarray(Wq, dtype=np.float32))
    wk = _tile_ed(np.asarray(Wk, dtype=np.float32))
    wv = _tile_ed(np.ascontiguousarray(np.asarray(Wv, np.float32).T))
    return xts, (wq, wk, wv)


def run(att_input, Wq, Wk, Wv, trace=False):
    xts, wts = _marshal(att_input, Wq, Wk, Wv)
    if trace:
        in_maps = [
            {"xt": xts[b], "wqn": wts[0], "wkn": wts[1], "wvt": wts[2]}
            for b in range(NB)
        ]
        res = bass_utils.run_bass_kernel_spmd(
            _get_nc(), in_maps, core_ids=list(range(NB)), trace=True
        )
        out = np.stack([res.results[b]["out"] for b in range(NB)], axis=0)
        return out.astype(np.float32, copy=False), res

    try:
        fn = _fast_runner()
        xs = xts.reshape(NB * P, DT, S)
        ws = [
            np.concatenate([w] * NB, axis=0).reshape(NB * P, DT, D)
            for w in wts
        ]
        zeros = np.zeros((NB * S, D), np.float32)
        (out,) = fn(xs, *ws, zeros)
        out = np.asarray(out)
    except Exception:
        in_maps = [
            {"xt": xts[b], "wqn": wts[0], "wkn": wts[1], "wvt": wts[2]}
            for b in range(NB)
        ]
        res = bass_utils.run_bass_kernel_spmd(
            _get_nc(), in_maps, core_ids=list(range(NB))
        )
        out = np.stack([res.results[b]["out"] for b in range(NB)], axis=0)
    return out.reshape(NB, S, D).astype(np.float32, copy=False), None


def kernel(att_input, Wq, Wk, Wv):
    out, _ = run(att_input, Wq, Wk, Wv)
    return out


# revision 14
# speedup vs baseline: 1.0612x; 1.0107x over previous
"""Single-head attention (B=8, S=2048, D=384) on 8 NeuronCores.

Sharding: data-parallel over batch — core b computes batch element b
entirely (the module is single-headed with no cross-batch coupling), all
three weight matrices replicated. Host marshalling is layout-only (zero
FLOPs): x pre-transposed/tiled to four contiguous [128, 3, 512] column
blocks, weights pre-tiled to [128, 3, 384] (Wq/Wk natural [e,d]; Wv
transposed [d,e]).

Per-core dataflow (f32 in/out, matmuls in float32r at full PE rate):
  - QK fold: scores = (x Wq^T)(x Wk^T)^T = x (Wq^T Wk) x^T. M = Wq^T Wk
    costs 9 tiny matmuls and replaces the separate Q and K projections
    with ONE projection TT = M^T x^T, saving ~15k PE cycles/core.
  - V = x Wv^T in natural [S, D] layout with two ones-columns appended.
  - alphaT[k, q] tiles = xT^T @ TT accumulated over 3 d-tiles; exp on
    ScalarE (|logit| < 60 so fp32 exp cannot overflow; softmax is
    shift-invariant so no max subtraction is needed).
  - PV and the softmax denominator accumulate together in one PSUM tile
    via the ones-columns (column D = sum_k exp); out = raw * recip(den).

Schedule (keyed to measured DMA behavior: ~12ns per contiguous run, so
only whole-tensor transfers with >=4KB per-partition runs are fast):
  - Head: wq rides the sync HWDGE ring (data ~8.2us, done ~10.6) and wk
    the scalar ring (done ~12.1) so M starts ~12.3us. x arrives as four
    whole-tensor column-block DMAs (x0+x3 on sync, x2 on scalar after
    wv, x1 on the SW DGE), all in by ~18us.
  - Warm-up: constants memset on GpSimdE (first engine out of the
    preamble), fp32 dummy matmuls from ~7.3us warm the HAM clock gate,
    f32r pads bridge to M.
  - chunk-0 attention interleaves the V projections (V(kt-1) at iter kt,
    consumed by PV(kt-3)) and the TT projections for chunks 1-3 (at kt
    5/9/12), so the PE never waits on the x DMA tail.
  - Tail: per-chunk epilogue fans out across engines (recips on VectorE,
    scales alternating VectorE/ScalarE, stores across sync/scalar HWDGE
    + gpsimd SW DGE) to keep the final chunk's epilogue short.
"""

import os
import numpy as np

import concourse.bacc as bacc
import concourse.tile as tile
from concourse import mybir
from concourse import bass_utils

P = 128
S = 2048
D = 384
NB = 8
DT = D // P
ST = S // P
QC = 512
NQ = S // QC
F32 = mybir.dt.float32
F32R = mybir.dt.float32r

N_WARM_F32 = int(os.environ.get("ATT_WARM_F32", "5"))
WARM_PRE = int(os.environ.get("ATT_WARM_PRE", "2"))
WARM_MID = int(os.environ.get("ATT_WARM_MID", "2"))
TT_KTS = (5, 9, 12)


def _build():
    nc = bacc.Bacc(
        "TRN2", target_bir_lowering=False, debug=False, enable_asserts=False
    )
    xts = [
        nc.dram_tensor(f"xt{i}", [P, DT, QC], F32R, kind="ExternalInput").ap()
        for i in range(NQ)
    ]
    wqn = nc.dram_tensor("wqn", [P, DT, D], F32R, kind="ExternalInput").ap()
    wkn = nc.dram_tensor("wkn", [P, DT, D], F32R, kind="ExternalInput").ap()
    wvt = nc.dram_tensor("wvt", [P, DT, D], F32R, kind="ExternalInput").ap()
    out = nc.dram_tensor("out", [S, D], F32, kind="ExternalOutput").ap()

    with tile.TileContext(nc) as tc:
        with (
            tc.tile_pool(name="const", bufs=1) as const_pool,
            tc.tile_pool(name="big", bufs=1) as big,
            tc.tile_pool(name="expool", bufs=5) as ex_pool,
            tc.tile_pool(name="obpool", bufs=4) as ob_pool,
            tc.tile_pool(name="smalls", bufs=4) as small_pool,
            tc.tile_pool(name="ps_stage", bufs=4, space="PSUM") as ps_stage,
            tc.tile_pool(name="ps_acc", bufs=4, space="PSUM") as ps_acc,
        ):
            xT = [
                big.tile([P, DT, QC], F32R, tag=f"xT{i}", name=f"xT{i}")
                for i in range(NQ)
            ]
            tT = big.tile([P, DT, S], F32R, tag="tT", name="tT")
            vA = big.tile([P, ST, D + 2], F32R, tag="vA", name="vA")
            wqT = big.tile([P, DT, D], F32R, tag="wqT", name="wqT")
            wkT = big.tile([P, DT, D], F32R, tag="wkT", name="wkT")
            wvT = big.tile([P, DT, D], F32R, tag="wvT", name="wvT")
            mT = big.tile([P, DT, D], F32R, tag="mT", name="mT")

            def xblk(kt):
                # x column block kt (128 cols) as [P, DT-slice, 128] views
                return xT[kt // 4], (kt % 4) * P

            # input DMAs first in per-engine program order; every transfer
            # is a whole contiguous tensor (>=4KB per-partition runs) since
            # ring throughput is ~12ns per contiguous run
            nc.sync.dma_start(out=wqT, in_=wqn)
            nc.sync.dma_start(out=xT[0], in_=xts[0])
            nc.sync.dma_start(out=xT[3], in_=xts[3])
            nc.scalar.dma_start(out=wkT, in_=wkn)
            nc.scalar.dma_start(out=wvT, in_=wvt)
            nc.scalar.dma_start(out=xT[2], in_=xts[2])

            # warm-up constants on GpSimdE (first engine out of the
            # framework preamble, ~5.9us); f32r tiles filled by DVE casts
            ones_c = const_pool.tile([P, 2], F32, tag="ones", name="ones_c")
            warm_z = const_pool.tile([P, QC], F32, tag="warmz", name="warm_z")
            warm_w = const_pool.tile([P, P], F32R, tag="warmw", name="warm_w")
            warm_m = const_pool.tile([P, QC], F32R, tag="warmm", name="warm_m")
            nc.gpsimd.memset(ones_c, 1.0)
            nc.gpsimd.memset(warm_z, 0.0)
            nc.gpsimd.dma_start(out=xT[1], in_=xts[1])
            nc.vector.tensor_copy(warm_m, warm_z)
            nc.vector.tensor_copy(warm_w, warm_z[:, 0:P])
            nc.vector.tensor_copy(
                vA[:, :, D:D + 2],
                ones_c.unsqueeze(1).broadcast_to([P, ST, 2]),
            )

            def proj_tile():
                return ps_stage.tile([P, QC], F32, tag="pj", name="pj")

            # PE prewarm: the HAM clock gate needs ~3.4us of sustained PE
            # activity to unthrottle 1.2 -> 2.4 GHz. fp32 dummies (ones x
            # zeros) start ~7.3us; f32r pads bridge to M.
            for _ in range(N_WARM_F32):
                pw = proj_tile()
                nc.tensor.matmul(
                    pw[0:2, :], ones_c, warm_z, start=True, stop=True
                )
            for _ in range(WARM_PRE):
                pw = proj_tile()
                nc.tensor.matmul(pw, warm_w, warm_m, start=True, stop=True)

            def compute_m():
                for dt_ in range(DT):
                    pm = proj_tile()
                    for et in range(DT):
                        nc.tensor.matmul(
                            pm[:, 0:D],
                            wqT[:, et, dt_ * P:(dt_ + 1) * P],
                            wkT[:, et, :],
                            start=(et == 0),
                            stop=(et == DT - 1),
                        )
                    nc.vector.tensor_copy(mT[:, dt_, :], pm[:, 0:D])

            def project_v(st):
                xtile, off = xblk(st)
                pv = proj_tile()
                for dt_ in range(DT):
                    nc.tensor.matmul(
                        pv[:, 0:D],
                        xtile[:, dt_, off:off + P],
                        wvT[:, dt_, :],
                        start=(dt_ == 0),
                        stop=(dt_ == DT - 1),
                    )
                nc.vector.tensor_copy(vA[:, st, 0:D], pv[:, 0:D])

            def project_t_chunk(qc, et):
                pp = proj_tile()
                for dt_ in range(DT):
                    nc.tensor.matmul(
                        pp,
                        mT[:, dt_, et * P:(et + 1) * P],
                        xT[qc][:, dt_, :],
                        start=(dt_ == 0),
                        stop=(dt_ == DT - 1),
                    )
                nc.vector.tensor_copy(tT[:, et, qc * QC:(qc + 1) * QC], pp)

            compute_m()
            for _ in range(WARM_MID):
                pw = proj_tile()
                nc.tensor.matmul(pw, warm_w, warm_m, start=True, stop=True)
            for et in range(DT):
                project_t_chunk(0, et)

            def emit_out(row, acc, slot):
                """Scale one 128-row output block and store it; engine and
                DMA queue rotate by slot so epilogues run concurrently."""
                rec = small_pool.tile([P, 1], F32, tag="rec", name="rec")
                ob = ob_pool.tile([P, D], F32, tag="ob", name="ob")
                nc.vector.reciprocal(rec, acc[:, D:D + 1])
                if slot % 2 == 0:
                    nc.vector.tensor_scalar_mul(ob, acc[:, 0:D], rec)
                else:
                    nc.scalar.activation(
                        ob,
                        acc[:, 0:D],
                        mybir.ActivationFunctionType.Copy,
                        scale=rec,
                    )
                if slot % 4 == 0:
                    nc.sync.dma_start(out=out[row:row + P, :], in_=ob)
                elif slot % 4 == 1:
                    nc.scalar.dma_start(out=out[row:row + P, :], in_=ob)
                elif slot % 4 == 2:
                    nc.gpsimd.dma_start(out=out[row:row + P, :], in_=ob)
                else:
                    nc.sync.dma_start(out=out[row:row + P, :], in_=ob)

            for c in range(NQ):
                accs = [
                    ps_acc.tile([P, D + 2], F32, tag="acc", name="acc")
                    for _ in range(4)
                ]

                def emit_pv(kt_i, ex):
                    for qs in range(4):
                        nc.tensor.matmul(
                            accs[qs],
                            ex[:, qs * P:(qs + 1) * P],
                            vA[:, kt_i, :],
                            start=(kt_i == 0),
                            stop=(kt_i == ST - 1),
                        )

                pending = []
                for kt_i in range(ST):
                    xtile, off = xblk(kt_i)
                    pa = ps_stage.tile([P, QC], F32, tag="pj", name="pa")
                    for et in range(DT):
                        nc.tensor.matmul(
                            pa,
                            xtile[:, et, off:off + P],
                            tT[:, et, c * QC:(c + 1) * QC],
                            start=(et == 0),
                            stop=(et == DT - 1),
                        )
                    ex = ex_pool.tile([P, QC], F32R, tag="ex", name="ex")
                    nc.scalar.activation(
                        ex, pa, mybir.ActivationFunctionType.Exp
                    )
                    pending.append((kt_i, ex))
                    if len(pending) > 3:
                        emit_pv(*pending.pop(0))
                    if c == 0:
                        if kt_i >= 1:
                            project_v(kt_i - 1)
                        if kt_i in TT_KTS:
                            qc = TT_KTS.index(kt_i) + 1
                            for et in range(DT):
                                project_t_chunk(qc, et)
                if c == 0:
                    project_v(ST - 1)
                for item in pending:
                    emit_pv(*item)

                for qs in range(4):
                    emit_out((c * 4 + qs) * P, accs[qs], qs)

    nc.compile()
    return nc


_NC = None
_FAST = None


def _get_nc():
    global _NC
    if _NC is None:
        _NC = _build()
    return _NC


IN_NAMES = ["xt0", "xt1", "xt2", "xt3", "wqn", "wkn", "wvt"]


def _fast_runner():
    global _FAST
    if _FAST is not None:
        return _FAST
    import jax
    from jax.experimental.shard_map import shard_map
    from jax.sharding import Mesh, PartitionSpec

    from concourse import bass2jax

    nc = _get_nc()
    bass2jax.install_neuronx_cc_hook()

    out_aval = jax.core.ShapedArray((S, D), np.float32)

    def _body(*args):
        operands = list(args)
        operands.append(bass2jax.partition_id_tensor())
        outs = bass2jax._bass_exec_p.bind(
            *operands,
            out_avals=(out_aval,),
            in_names=tuple(IN_NAMES) + ("out", "partition_id"),
            out_names=("out",),
            lowering_input_output_aliases=(),
            sim_require_finite=True,
            sim_require_nnan=True,
            nc=nc,
        )
        return tuple(outs)

    devices = jax.devices()[:NB]
    mesh = Mesh(np.asarray(devices), ("core",))
    n_in = len(IN_NAMES) + 1
    fn = jax.jit(
        shard_map(
            _body,
            mesh=mesh,
            in_specs=(PartitionSpec("core"),) * n_in,
            out_specs=(PartitionSpec("core"),),
            check_rep=False,
        ),
        donate_argnums=(n_in - 1,),
        keep_unused=True,
    )
    _FAST = fn
    return fn


def _tile_ed(w):
    return np.ascontiguousarray(
        w.reshape(DT, P, w.shape[1]).transpose(1, 0, 2)
    )


def _marshal(att_input, Wq, Wk, Wv):
    att_input = np.asarray(att_input, dtype=np.float32)
    xts = np.ascontiguousarray(
        att_input.transpose(0, 2, 1)
        .reshape(NB, DT, P, S)
        .transpose(0, 2, 1, 3)
    )
    xparts = [
        np.ascontiguousarray(xts[:, :, :, i * QC:(i + 1) * QC])
        for i in range(NQ)
    ]
    wq = _tile_ed(np.asarray(Wq, dtype=np.float32))
    wk = _tile_ed(np.asarray(Wk, dtype=np.float32))
    wv = _tile_ed(np.ascontiguousarray(np.asarray(Wv, np.float32).T))
    return xparts, (wq, wk, wv)


def run(att_input, Wq, Wk, Wv, trace=False):
    xparts, wts = _marshal(att_input, Wq, Wk, Wv)
    if trace:
        in_maps = [
            {
                "xt0": xparts[0][b],
                "xt1": xparts[1][b],
                "xt2": xparts[2][b],
                "xt3": xparts[3][b],
                "wqn": wts[0],
                "wkn": wts[1],
                "wvt": wts[2],
            }
            for b in range(NB)
        ]
        res = bass_utils.run_bass_kernel_spmd(
            _get_nc(), in_maps, core_ids=list(range(NB)), trace=True
        )
        out = np.stack([res.results[b]["out"] for b in range(NB)], axis=0)
        return out.astype(np.float32, copy=False), res

    try:
        fn = _fast_runner()
        xs = [x.reshape(NB * P, DT, QC) for x in xparts]
        ws = [
            np.concatenate([w] * NB, axis=0).reshape(NB * P, DT, D)
            for w in wts
        ]
        zeros = np.zeros((NB * S, D), np.float32)
        (out,) = fn(*xs, *ws, zeros)
        out = np.asarray(out)
    except Exception:
        in_maps = [
            {
                "xt0": xparts[0][b],
                "xt1": xparts[1][b],
                "xt2": xparts[2][b],
                "xt3": xparts[3][b],
                "wqn": wts[0],
                "wkn": wts[1],
                "wvt": wts[2],
            }
            for b in range(NB)
        ]
        res = bass_utils.run_bass_kernel_spmd(
            _get_nc(), in_maps, core_ids=list(range(NB))
        )
        out = np.stack([res.results[b]["out"] for b in range(NB)], axis=0)
    return out.reshape(NB, S, D).astype(np.float32, copy=False), None


def kernel(att_input, Wq, Wk, Wv):
    out, _ = run(att_input, Wq, Wk, Wv)
    return out


# revision 16
# speedup vs baseline: 1.0999x; 1.0365x over previous
"""Single-head attention (B=8, S=2048, D=384) on 8 NeuronCores.

Sharding: data-parallel over batch — core b computes batch element b
entirely (the module is single-headed with no cross-batch coupling), all
three weight matrices replicated. Host marshalling is layout-only (zero
FLOPs): x pre-transposed/tiled to four contiguous [128, 3, 512] column
blocks, weights pre-tiled to [128, 3, 384] (Wq/Wk natural [e,d]; Wv
transposed [d,e]).

Per-core dataflow (f32 in/out, matmuls in float32r at full PE rate):
  - QK fold: scores = (x Wq^T)(x Wk^T)^T = x (Wq^T Wk) x^T. M = Wq^T Wk
    costs 9 tiny matmuls and replaces the separate Q and K projections
    with ONE projection TT = M^T x^T, saving ~15k PE cycles/core.
  - V = x Wv^T in natural [S, D] layout with two ones-columns appended.
  - alphaT[k, q] tiles = xT^T @ TT accumulated over 3 d-tiles; exp on
    ScalarE (|logit| < 60 so fp32 exp cannot overflow; softmax is
    shift-invariant so no max subtraction is needed).
  - PV and the softmax denominator accumulate together in one PSUM tile
    via the ones-columns (column D = sum_k exp); out = raw * recip(den).

Schedule (keyed to measured DMA behavior: ~12ns per contiguous run, so
only whole-tensor transfers with >=4KB per-partition runs are fast):
  - Head: wq rides the sync HWDGE ring (data ~8.2us, done ~10.6) and wk
    the scalar ring (done ~12.1) so M starts ~12.3us. x arrives as four
    whole-tensor column-block DMAs (x0+x3 on sync, x2 on scalar after
    wv, x1 on the SW DGE), all in by ~18us.
  - Warm-up: constants memset on GpSimdE (first engine out of the
    preamble), fp32 dummy matmuls from ~7.3us warm the HAM clock gate,
    f32r pads bridge to M.
  - chunk-0 attention interleaves the V projections (V(kt-1) at iter kt,
    consumed by PV(kt-3)) and the TT projections for chunks 1-3 (at kt
    5/9/12), so the PE never waits on the x DMA tail.
  - Tail: per-chunk epilogue fans out across engines (recips on VectorE,
    scales alternating VectorE/ScalarE, stores across sync/scalar HWDGE
    + gpsimd SW DGE) to keep the final chunk's epilogue short.
"""

import os
import numpy as np

import concourse.bacc as bacc
import concourse.tile as tile
from concourse import mybir
from concourse import bass_utils

P = 128
S = 2048
D = 384
NB = 8
DT = D // P
ST = S // P
QC = 512
NQ = S // QC
F32 = mybir.dt.float32
F32R = mybir.dt.float32r

N_WARM_F32 = int(os.environ.get("ATT_WARM_F32", "5"))
WARM_PRE = int(os.environ.get("ATT_WARM_PRE", "2"))
WARM_MID = int(os.environ.get("ATT_WARM_MID", "2"))
TT_KTS = (5, 9, 12)
# attention q-chunks: (col_lo, width). The final two 256-wide chunks keep
# the very last epilogue small (2 blocks over both HWDGE rings) without
# making exp tiles so narrow that ScalarE becomes the bottleneck.
CHUNKS = ((0, 512), (512, 512), (1024, 512), (1536, 256), (1792, 256))


def _build():
    nc = bacc.Bacc(
        "TRN2", target_bir_lowering=False, debug=False, enable_asserts=False
    )
    xts = [
        nc.dram_tensor(f"xt{i}", [P, DT, QC], F32R, kind="ExternalInput").ap()
        for i in range(NQ)
    ]
    wqn = nc.dram_tensor("wqn", [P, DT, D], F32R, kind="ExternalInput").ap()
    wkn = nc.dram_tensor("wkn", [P, DT, D], F32R, kind="ExternalInput").ap()
    wvt = nc.dram_tensor("wvt", [P, DT, D], F32R, kind="ExternalInput").ap()
    out = nc.dram_tensor("out", [S, D], F32, kind="ExternalOutput").ap()

    with tile.TileContext(nc) as tc:
        with (
            tc.tile_pool(name="const", bufs=1) as const_pool,
            tc.tile_pool(name="big", bufs=1) as big,
            tc.tile_pool(name="expool", bufs=5) as ex_pool,
            tc.tile_pool(name="obpool", bufs=4) as ob_pool,
            tc.tile_pool(name="smalls", bufs=4) as small_pool,
            tc.tile_pool(name="ps_stage", bufs=4, space="PSUM") as ps_stage,
            tc.tile_pool(name="ps_acc", bufs=4, space="PSUM") as ps_acc,
        ):
            xT = [
                big.tile([P, DT, QC], F32R, tag=f"xT{i}", name=f"xT{i}")
                for i in range(NQ)
            ]
            tT = big.tile([P, DT, S], F32R, tag="tT", name="tT")
            vA = big.tile([P, ST, D + 2], F32R, tag="vA", name="vA")
            wqT = big.tile([P, DT, D], F32R, tag="wqT", name="wqT")
            wkT = big.tile([P, DT, D], F32R, tag="wkT", name="wkT")
            wvT = big.tile([P, DT, D], F32R, tag="wvT", name="wvT")
            mT = big.tile([P, DT, D], F32R, tag="mT", name="mT")

            def xblk(kt):
                # x column block kt (128 cols) as [P, DT-slice, 128] views
                return xT[kt // 4], (kt % 4) * P

            # input DMAs first in per-engine program order; every transfer
            # is a whole contiguous tensor (>=4KB per-partition runs) since
            # ring throughput is ~12ns per contiguous run
            nc.sync.dma_start(out=wqT, in_=wqn)
            nc.sync.dma_start(out=xT[0], in_=xts[0])
            nc.sync.dma_start(out=xT[3], in_=xts[3])
            nc.scalar.dma_start(out=wkT, in_=wkn)
            nc.scalar.dma_start(out=wvT, in_=wvt)
            nc.scalar.dma_start(out=xT[1], in_=xts[1])
            nc.scalar.dma_start(out=xT[2], in_=xts[2])

            # warm-up constants on GpSimdE (first engine out of the
            # framework preamble, ~5.9us); f32r tiles filled by DVE casts
            ones_c = const_pool.tile([P, 2], F32, tag="ones", name="ones_c")
            warm_z = const_pool.tile([P, QC], F32, tag="warmz", name="warm_z")
            warm_w = const_pool.tile([P, P], F32R, tag="warmw", name="warm_w")
            warm_m = const_pool.tile([P, QC], F32R, tag="warmm", name="warm_m")
            nc.gpsimd.memset(ones_c, 1.0)
            nc.gpsimd.memset(warm_z, 0.0)
            nc.vector.tensor_copy(warm_m, warm_z)
            nc.vector.tensor_copy(warm_w, warm_z[:, 0:P])
            nc.vector.tensor_copy(
                vA[:, :, D:D + 2],
                ones_c.unsqueeze(1).broadcast_to([P, ST, 2]),
            )

            def proj_tile():
                return ps_stage.tile([P, QC], F32, tag="pj", name="pj")

            # PE prewarm: the HAM clock gate needs ~3.4us of sustained PE
            # activity to unthrottle 1.2 -> 2.4 GHz. fp32 dummies (ones x
            # zeros) start ~7.3us; f32r pads bridge to M.
            for _ in range(N_WARM_F32):
                pw = proj_tile()
                nc.tensor.matmul(
                    pw[0:2, :], ones_c, warm_z, start=True, stop=True
                )
            for _ in range(WARM_PRE):
                pw = proj_tile()
                nc.tensor.matmul(pw, warm_w, warm_m, start=True, stop=True)

            def compute_m():
                for dt_ in range(DT):
                    pm = proj_tile()
                    for et in range(DT):
                        nc.tensor.matmul(
                            pm[:, 0:D],
                            wqT[:, et, dt_ * P:(dt_ + 1) * P],
                            wkT[:, et, :],
                            start=(et == 0),
                            stop=(et == DT - 1),
                        )
                    nc.vector.tensor_copy(mT[:, dt_, :], pm[:, 0:D])

            def project_v(st):
                xtile, off = xblk(st)
                pv = proj_tile()
                for dt_ in range(DT):
                    nc.tensor.matmul(
                        pv[:, 0:D],
                        xtile[:, dt_, off:off + P],
                        wvT[:, dt_, :],
                        start=(dt_ == 0),
                        stop=(dt_ == DT - 1),
                    )
                nc.vector.tensor_copy(vA[:, st, 0:D], pv[:, 0:D])

            def project_t_chunk(qc, et):
                pp = proj_tile()
                for dt_ in range(DT):
                    nc.tensor.matmul(
                        pp,
                        mT[:, dt_, et * P:(et + 1) * P],
                        xT[qc][:, dt_, :],
                        start=(dt_ == 0),
                        stop=(dt_ == DT - 1),
                    )
                nc.vector.tensor_copy(tT[:, et, qc * QC:(qc + 1) * QC], pp)

            compute_m()
            for _ in range(WARM_MID):
                pw = proj_tile()
                nc.tensor.matmul(pw, warm_w, warm_m, start=True, stop=True)
            for et in range(DT):
                project_t_chunk(0, et)

            def emit_out(row, acc, slot, final=False):
                """Scale one 128-row output block and store it; engine and
                DMA queue rotate by slot so epilogues run concurrently. The
                final chunk's two blocks use one HWDGE ring each (never the
                slow SW DGE) to keep the kernel tail short."""
                rec = small_pool.tile([P, 1], F32, tag="rec", name="rec")
                ob = ob_pool.tile([P, D], F32, tag="ob", name="ob")
                nc.vector.reciprocal(rec, acc[:, D:D + 1])
                if slot % 2 == 0:
                    nc.vector.tensor_scalar_mul(ob, acc[:, 0:D], rec)
                else:
                    nc.scalar.activation(
                        ob,
                        acc[:, 0:D],
                        mybir.ActivationFunctionType.Copy,
                        scale=rec,
                    )
                if final:
                    if slot % 2 == 0:
                        nc.sync.dma_start(out=out[row:row + P, :], in_=ob)
                    else:
                        nc.scalar.dma_start(out=out[row:row + P, :], in_=ob)
                elif slot % 4 == 0:
                    nc.sync.dma_start(out=out[row:row + P, :], in_=ob)
                elif slot % 4 == 1:
                    nc.scalar.dma_start(out=out[row:row + P, :], in_=ob)
                elif slot % 4 == 2:
                    nc.gpsimd.dma_start(out=out[row:row + P, :], in_=ob)
                else:
                    nc.sync.dma_start(out=out[row:row + P, :], in_=ob)

            slot_n = [0]
            for ci, (qlo, qw) in enumerate(CHUNKS):
                nblk = qw // P
                accs = [
                    ps_acc.tile([P, D + 2], F32, tag="acc", name="acc")
                    for _ in range(nblk)
                ]

                def emit_pv(kt_i, ex):
                    for qs in range(nblk):
                        nc.tensor.matmul(
                            accs[qs],
                            ex[:, qs * P:(qs + 1) * P],
                            vA[:, kt_i, :],
                            start=(kt_i == 0),
                            stop=(kt_i == ST - 1),
                        )

                pending = []
                for kt_i in range(ST):
                    xtile, off = xblk(kt_i)
                    pa = ps_stage.tile([P, QC], F32, tag="pj", name="pa")
                    for et in range(DT):
                        nc.tensor.matmul(
                            pa[:, 0:qw],
                            xtile[:, et, off:off + P],
                            tT[:, et, qlo:qlo + qw],
                            start=(et == 0),
                            stop=(et == DT - 1),
                        )
                    ex = ex_pool.tile([P, QC], F32R, tag="ex", name="ex")
                    nc.scalar.activation(
                        ex[:, 0:qw], pa[:, 0:qw],
                        mybir.ActivationFunctionType.Exp,
                    )
                    pending.append((kt_i, ex))
                    if len(pending) > 3:
                        emit_pv(*pending.pop(0))
                    if ci == 0:
                        if kt_i >= 1:
                            project_v(kt_i - 1)
                        if kt_i in TT_KTS:
                            qc = TT_KTS.index(kt_i) + 1
                            for et in range(DT):
                                project_t_chunk(qc, et)
                if ci == 0:
                    project_v(ST - 1)
                for item in pending:
                    emit_pv(*item)

                is_last = ci == len(CHUNKS) - 1
                for qs in range(nblk):
                    emit_out(qlo + qs * P, accs[qs], slot_n[0], final=is_last)
                    slot_n[0] += 1

    nc.compile()
    return nc


_NC = None
_FAST = None


def _get_nc():
    global _NC
    if _NC is None:
        _NC = _build()
    return _NC


IN_NAMES = ["xt0", "xt1", "xt2", "xt3", "wqn", "wkn", "wvt"]


def _fast_runner():
    global _FAST
    if _FAST is not None:
        return _FAST
    import jax
    from jax.experimental.shard_map import shard_map
    from jax.sharding import Mesh, PartitionSpec

    from concourse import bass2jax

    nc = _get_nc()
    bass2jax.install_neuronx_cc_hook()

    out_aval = jax.core.ShapedArray((S, D), np.float32)

    def _body(*args):
        operands = list(args)
        operands.append(bass2jax.partition_id_tensor())
        outs = bass2jax._bass_exec_p.bind(
            *operands,
            out_avals=(out_aval,),
            in_names=tuple(IN_NAMES) + ("out", "partition_id"),
            out_names=("out",),
            lowering_input_output_aliases=(),
            sim_require_finite=True,
            sim_require_nnan=True,
            nc=nc,
        )
        return tuple(outs)

    devices = jax.devices()[:NB]
    mesh = Mesh(np.asarray(devices), ("core",))
    n_in = len(IN_NAMES) + 1
    fn = jax.jit(
        shard_map(
            _body,
            mesh=mesh,
            in_specs=(PartitionSpec("core"),) * n_in,
            out_specs=(PartitionSpec("core"),),
            check_rep=False,
        ),
        donate_argnums=(n_in - 1,),
        keep_unused=True,
    )
    _FAST = fn
    return fn


def _tile_ed(w):
    return np.ascontiguousarray(
        w.reshape(DT, P, w.shape[1]).transpose(1, 0, 2)
    )


def _marshal(att_input, Wq, Wk, Wv):
    att_input = np.asarray(att_input, dtype=np.float32)
    xts = np.ascontiguousarray(
        att_input.transpose(0, 2, 1)
        .reshape(NB, DT, P, S)
        .transpose(0, 2, 1, 3)
    )
    xparts = [
        np.ascontiguousarray(xts[:, :, :, i * QC:(i + 1) * QC])
        for i in range(NQ)
    ]
    wq = _tile_ed(np.asarray(Wq, dtype=np.float32))
    wk = _tile_ed(np.asarray(Wk, dtype=np.float32))
    wv = _tile_ed(np.ascontiguousarray(np.asarray(Wv, np.float32).T))
    return xparts, (wq, wk, wv)


def run(att_input, Wq, Wk, Wv, trace=False):
    xparts, wts = _marshal(att_input, Wq, Wk, Wv)
    if trace:
        in_maps = [
            {
                "xt0": xparts[0][b],
                "xt1": xparts[1][b],
                "xt2": xparts[2][b],
                "xt3": xparts[3][b],
                "wqn": wts[0],
                "wkn": wts[1],
                "wvt": wts[2],
            }
            for b in range(NB)
        ]
        res = bass_utils.run_bass_kernel_spmd(
            _get_nc(), in_maps, core_ids=list(range(NB)), trace=True
        )
        out = np.stack([res.results[b]["out"] for b in range(NB)], axis=0)
        return out.astype(np.float32, copy=False), res

    try:
        fn = _fast_runner()
        xs = [x.reshape(NB * P, DT, QC) for x in xparts]
        ws = [
            np.concatenate([w] * NB, axis=0).reshape(NB * P, DT, D)
            for w in wts
        ]
        zeros = np.zeros((NB * S, D), np.float32)
        (out,) = fn(*xs, *ws, zeros)
        out = np.asarray(out)
    except Exception:
        in_maps = [
            {
                "xt0": xparts[0][b],
                "xt1": xparts[1][b],
                "xt2": xparts[2][b],
                "xt3": xparts[3][b],
                "wqn": wts[0],
                "wkn": wts[1],
                "wvt": wts[2],
            }
            for b in range(NB)
        ]
        res = bass_utils.run_bass_kernel_spmd(
            _get_nc(), in_maps, core_ids=list(range(NB))
        )
        out = np.stack([res.results[b]["out"] for b in range(NB)], axis=0)
    return out.reshape(NB, S, D).astype(np.float32, copy=False), None


def kernel(att_input, Wq, Wk, Wv):
    out, _ = run(att_input, Wq, Wk, Wv)
    return out
